# revision 43
# baseline (speedup 1.0000x reference)
"""DenseGAT layer (top-16 sparsified, 4 heads) as a Bass/Tile kernel on 8
Trainium2 NeuronCores.

Sharding: 1D row partition of i (the destination-node axis). Each core gets a
512-row slab of adj and of x. Per core:
  phase 1: project own x slab through augmented weights [W.T | w_src | w_dst]
           -> rows [Wh(512) | s_src(4) | s_dst(4)] in bf16; AllGather -> full
           4096x520 bf16 table in shared DRAM.
  phase 2 (per 128-row tile): top-16 of adj row (DVE max/max_index/
           match_replace, exact jax tie-break, fp32), indirect-DMA gather of
           the 16 neighbor rows (bf16, spread over 4 SWDGE queues), leaky-relu
           scores + softmax over 16 (fp32 DVE+ACT), then the weighted sum as
           one in-place GpSimd multiply (alpha broadcast via stride-0 AP,
           bf16) plus a k-reduction on PE (16 PSUM-accumulated identity
           matmuls), ELU in fp32, bf16 store.

bf16 is used for the x/W/a inputs, the projection, the all-gathered table,
the gather, the weighted sum, and the output: it halves input/output staging,
collective flight, and gather bytes (the dominant per-call costs — measured:
4 SWDGE queues instead of 1 cut the post-collective gather tail by ~0.6 ms,
and bf16 cut the rest). adj and the top-k selection stay fp32 so the top-16
set matches jax exactly; softmax/ELU math stays fp32. Overall rel err vs the
fp32 reference is ~4e-3 (gate: 2e-2).

kernel(**inputs) takes FULL fp32 inputs and returns the FULL (4096, 512) fp32
output.
"""
import os
import sys

sys.path.insert(0, "/opt/trn_rl_repo")

import numpy as np
from ml_dtypes import bfloat16

import concourse.bass as bass
import concourse.bacc as bacc
import concourse.mybir as mybir
from concourse.tile import TileContext
from concourse.bass_utils import run_bass_kernel_spmd
from concourse.masks import make_identity

NCORES = 8
N = 4096
DIN = 1024
DOUT = 512
H = 4
DH = 128
K = 16
NS = N // NCORES          # 512 rows per core
T = NS // 128             # 4 tiles of 128 rows per core
AUG = DOUT + 2 * H        # 520: [Wh | s_src | s_dst]
NEG_SLOPE = 0.2
FP = mybir.dt.float32
BF = mybir.dt.bfloat16


def build_program():
    nq = int(os.environ.get("KNL_SWDGE_QUEUES", "4"))
    nc = bacc.Bacc(
        "TRN2",
        target_bir_lowering=False,
        debug=False,
        num_devices=NCORES,
        num_swdge_queues=nq,
    )

    x_s = nc.dram_tensor("x_s", [NS, DIN], BF, kind="ExternalInput")
    adj_s = nc.dram_tensor("adj_s", [NS, N], FP, kind="ExternalInput")
    W = nc.dram_tensor("W", [DOUT, DIN], BF, kind="ExternalInput")
    a = nc.dram_tensor("a", [1, 2 * DH], BF, kind="ExternalInput")
    out_s = nc.dram_tensor("out_s", [NS, DOUT], BF, kind="ExternalOutput")

    whs_full = nc.dram_tensor("whs_full", [N, AUG], BF, addr_space="Shared")

    with TileContext(nc) as tc:
        with (
            tc.tile_pool(name="const", bufs=1) as cpool,
            tc.tile_pool(name="dram", bufs=1, space="DRAM") as dpool,
        ):
            ident = cpool.tile([128, 128], BF)
            make_identity(nc, ident[:])

            # ---------------- phase 1: augmented projection ----------------
            own_si = cpool.tile([128, T, H], BF)   # s_src of own rows
            whs_slab = dpool.tile([NS, AUG], BF)

            with (
                tc.tile_pool(name="p1", bufs=1) as p1,
                tc.tile_pool(name="p1ps", bufs=2, space="PSUM") as p1ps,
                tc.tile_pool(name="p1ps_small", bufs=2, space="PSUM") as p1ps_s,
            ):
                wsb = p1.tile([128, H, DIN], BF)       # W[h*128+p, d]
                nc.sync.dma_start(wsb[:], W.rearrange("(h p) d -> p h d", p=128))
                a_sb = p1.tile([128, 2], BF)           # a_src | a_dst by dh
                nc.sync.dma_start(
                    a_sb[:], a[0:1, :].rearrange("o (s p) -> p (o s)", p=128)
                )
                xsb = p1.tile([128, T, DIN], BF)       # x rows t*128+p
                nc.sync.dma_start(xsb[:], x_s.rearrange("(t p) d -> p t d", p=128))

                # aug_rhs[:, c*AUG : c*AUG+512] = W.T chunk c; cols 512+h / 516+h
                # = w_src_h / w_dst_h (a pre-contracted with W).
                aug_rhs = p1.tile([128, 8, AUG], BF)

                for h in range(H):
                    for c in range(8):
                        ps = p1ps_s.tile([128, 2], FP, tag="wsd")
                        nc.tensor.matmul(
                            out=ps[:],
                            lhsT=wsb[:, h, c * 128 : (c + 1) * 128],
                            rhs=a_sb[:],
                            start=True,
                            stop=True,
                        )
                        # cols 512+h (src) and 516+h (dst) of chunk c
                        dst = aug_rhs[:, c, DOUT + h : DOUT + h + 5 : 4]
                        nc.vector.tensor_copy(dst, ps[:])

                for c in range(8):
                    pst = p1ps.tile([128, 512], BF, tag="tp")
                    for h in range(H):
                        nc.tensor.transpose(
                            out=pst[:, h * 128 : (h + 1) * 128],
                            in_=wsb[:, h, c * 128 : (c + 1) * 128],
                            identity=ident[:],
                        )
                    nc.vector.tensor_copy(aug_rhs[:, c, 0:DOUT], pst[:])

                xT = p1.tile([128, T * 8, 128], BF)   # x.T chunks [d, i]
                for t in range(T):
                    for g in range(2):
                        pst = p1ps.tile([128, 512], BF, tag="tp")
                        for j in range(4):
                            c = g * 4 + j
                            nc.tensor.transpose(
                                out=pst[:, j * 128 : (j + 1) * 128],
                                in_=xsb[:, t, c * 128 : (c + 1) * 128],
                                identity=ident[:],
                            )
                        nc.vector.tensor_copy(
                            xT[:, t * 8 + g * 4 : t * 8 + g * 4 + 4, :], pst[:]
                        )

                for t in range(T):
                    psA = p1ps.tile([128, 512], FP, tag="proj")
                    psB = p1ps_s.tile([128, 8], FP, tag="projb")
                    for c in range(8):
                        nc.tensor.matmul(
                            out=psA[:],
                            lhsT=xT[:, t * 8 + c, :],
                            rhs=aug_rhs[:, c, 0:DOUT],
                            start=(c == 0),
                            stop=(c == 7),
                        )
                        nc.tensor.matmul(
                            out=psB[:],
                            lhsT=xT[:, t * 8 + c, :],
                            rhs=aug_rhs[:, c, DOUT:AUG],
                            start=(c == 0),
                            stop=(c == 7),
                        )
                    whs_t = p1.tile([128, AUG], BF, tag="whs")
                    nc.vector.tensor_copy(whs_t[:, 0:DOUT], psA[:])
                    nc.vector.tensor_copy(whs_t[:, DOUT:AUG], psB[:])
                    nc.vector.tensor_copy(own_si[:, t, :], psB[:, 0:H])
                    nc.sync.dma_start(
                        whs_slab[t * 128 : (t + 1) * 128, :], whs_t[:]
                    )

            if not os.environ.get("KNL_NO_CC"):
                nc.gpsimd.collective_compute(
                    "AllGather",
                    mybir.AluOpType.bypass,
                    replica_groups=[list(range(NCORES))],
                    ins=[whs_slab[:]],
                    outs=[whs_full[:]],
                )

            # ---------------- phase 2: per-tile topk/softmax/fma ----------------
            # (Tried hoisting all tiles' top-k ahead of the gathers so the
            # DVE scans fully overlap the AllGather — the tile scheduler
            # re-interleaved the streams and the modeled span got ~28µs
            # WORSE, so the single-pass per-tile form stays.)
            # (Scheduler-tuning attempts that did NOT help, per the 8-core
            # timing model: hoisting all top-k ahead of the gathers
            # (236→264µs) and bufs=4 on gp/smallp (236→242µs). The tile
            # scheduler's stream ordering doesn't respond predictably to
            # emission order or buffer depth; this configuration measured
            # best on HW and in the model.)
            gp_bufs = int(os.environ.get("KNL_GP_BUFS", "2"))
            small_bufs = int(os.environ.get("KNL_SMALL_BUFS", "2"))
            with (
                tc.tile_pool(name="adjp", bufs=2) as adjp,
                tc.tile_pool(name="gp", bufs=gp_bufs) as gp,
                tc.tile_pool(name="smallp", bufs=small_bufs) as smallp,
                tc.tile_pool(name="outp", bufs=2) as outp,
                tc.tile_pool(name="accp", bufs=2, space="PSUM") as accp,
            ):
                for t in range(T):
                    adj_t = adjp.tile([128, N], FP, tag="adj")
                    nc.sync.dma_start(adj_t[:], adj_s[t * 128 : (t + 1) * 128, :])

                    m8a = smallp.tile([128, 8], FP, tag="m8a")
                    m8b = smallp.tile([128, 8], FP, tag="m8b")
                    idx = smallp.tile([128, K], mybir.dt.uint32, tag="idx")
                    nc.vector.max(out=m8a[:], in_=adj_t[:])
                    nc.vector.max_index(out=idx[:, 0:8], in_max=m8a[:], in_values=adj_t[:])
                    nc.vector.match_replace(
                        out=adj_t[:], in_to_replace=m8a[:], in_values=adj_t[:],
                        imm_value=-1.0,
                    )
                    nc.vector.max(out=m8b[:], in_=adj_t[:])
                    nc.vector.max_index(out=idx[:, 8:16], in_max=m8b[:], in_values=adj_t[:])

                    G = gp.tile([128, K, AUG], BF, tag="G")
                    if os.environ.get("KNL_NO_GATHER"):
                        nc.vector.memset(G[:, 0, :], 0.5)
                    else:
                        # one indirect DMA per k: a single multi-offset
                        # instruction (idx[:, 0:K]) simulates correctly but
                        # crashes the NEFF at exec on this toolchain, like
                        # dma_gather — keep the per-k form.
                        for k in range(K):
                            nc.gpsimd.indirect_dma_start(
                                out=G[:, k, :],
                                out_offset=None,
                                in_=whs_full[:],
                                in_offset=bass.IndirectOffsetOnAxis(
                                    ap=idx[:, k : k + 1], axis=0
                                ),
                            )

                    # scores: e[p, h, k] = leaky(s_i[p,h] + s_dst[idx[p,k], h])
                    S = smallp.tile([128, H, K], FP, tag="S")
                    nc.vector.tensor_tensor(
                        out=S[:],
                        in0=G[:, :, DOUT + H : AUG].rearrange("p k h -> p h k"),
                        in1=own_si[:, t, :].to_broadcast([128, H, K]),
                        op=mybir.AluOpType.add,
                    )
                    E = smallp.tile([128, H, K], FP, tag="E")
                    nc.vector.scalar_tensor_tensor(
                        out=E[:],
                        in0=S[:],
                        scalar=NEG_SLOPE,
                        in1=S[:],
                        op0=mybir.AluOpType.mult,
                        op1=mybir.AluOpType.max,
                    )
                    M = smallp.tile([128, H], FP, tag="M")
                    nc.vector.tensor_reduce(
                        out=M[:], in_=E[:], axis=mybir.AxisListType.X,
                        op=mybir.AluOpType.max,
                    )
                    negM = smallp.tile([128, H], FP, tag="negM")
                    nc.vector.tensor_scalar(
                        out=negM[:], in0=M[:], scalar1=-1.0, scalar2=None,
                        op0=mybir.AluOpType.mult,
                    )
                    P = smallp.tile([128, H, K], FP, tag="P")
                    Z = smallp.tile([128, H], FP, tag="Z")
                    for h in range(H):
                        nc.scalar.activation(
                            out=P[:, h, :],
                            in_=E[:, h, :],
                            func=mybir.ActivationFunctionType.Exp,
                            bias=negM[:, h : h + 1],
                            scale=1.0,
                            accum_out=Z[:, h : h + 1],
                        )
                    rec = smallp.tile([128, H], FP, tag="rec")
                    nc.vector.reciprocal(out=rec[:], in_=Z[:])
                    A = smallp.tile([128, H, K], BF, tag="A")
                    for h in range(H):
                        nc.vector.tensor_scalar(
                            out=A[:, h, :], in0=P[:, h, :],
                            scalar1=rec[:, h : h + 1], scalar2=None,
                            op0=mybir.AluOpType.mult,
                        )

                    # weighted sum: scale gathered Wh rows by alpha in place
                    # (one big op, alpha broadcast along c via stride-0),
                    # then reduce over k (strided, k innermost).
                    # alpha-scale stays on GpSimd/Pool: moving it to DVE
                    # (modeled 258µs) or half-half (254µs) is worse than
                    # all-Pool (236µs) — DVE is the schedule-critical engine
                    # in the tail, and the model gives it no bf16 2× credit.
                    gview = G[:, :, 0:DOUT].rearrange("p k (h c) -> p k h c", h=H)
                    nc.gpsimd.tensor_tensor(
                        out=gview,
                        in0=gview,
                        in1=A[:].rearrange("p h k -> p k h").to_broadcast([128, K, H, DH]),
                        op=mybir.AluOpType.mult,
                    )
                    # k-reduction on PE (idle in phase 2): identity-weight
                    # matmuls accumulating the 16 scaled rows in one PSUM bank.
                    osum = accp.tile([128, DOUT], FP, tag="acc")
                    for k in range(K):
                        nc.tensor.matmul(
                            out=osum[:],
                            lhsT=ident[:],
                            rhs=G[:, k, 0:DOUT],
                            start=(k == 0),
                            stop=(k == K - 1),
                        )

                    # elu(x) = relu(x) + exp(min(x,0)) - 1
                    u = outp.tile([128, DOUT], FP, tag="u")
                    nc.vector.tensor_scalar(
                        out=u[:], in0=osum[:], scalar1=0.0, scalar2=None,
                        op0=mybir.AluOpType.min,
                    )
                    e1 = outp.tile([128, DOUT], FP, tag="e1")
                    nc.scalar.activation(
                        out=e1[:], in_=u[:], func=mybir.ActivationFunctionType.Exp,
                    )
                    r1 = outp.tile([128, DOUT], FP, tag="r1")
                    nc.scalar.activation(
                        out=r1[:], in_=osum[:], func=mybir.ActivationFunctionType.Relu,
                    )
                    o = outp.tile([128, DOUT], BF, tag="o")
                    nc.vector.scalar_tensor_tensor(
                        out=o[:], in0=e1[:], scalar=-1.0, in1=r1[:],
                        op0=mybir.AluOpType.add, op1=mybir.AluOpType.add,
                    )
                    nc.sync.dma_start(out_s[t * 128 : (t + 1) * 128, :], o[:])

    nc.compile()
    return nc


_NC_CACHE = None


def _get_program():
    global _NC_CACHE
    if _NC_CACHE is None:
        _NC_CACHE = build_program()
    return _NC_CACHE


def make_in_maps(x, adj, W, a):
    """Per-core input maps; converts x/W/a to bf16 (must match the program's
    declared input dtypes)."""
    x = np.asarray(x, dtype=np.float32).astype(bfloat16)
    adj = np.ascontiguousarray(np.asarray(adj, dtype=np.float32))
    W = np.asarray(W, dtype=np.float32).astype(bfloat16)
    a = np.asarray(a, dtype=np.float32).astype(bfloat16)
    return [
        {
            "x_s": x[c * NS : (c + 1) * NS],
            "adj_s": adj[c * NS : (c + 1) * NS],
            "W": W,
            "a": a,
        }
        for c in range(NCORES)
    ]


def kernel(x, adj, W, a, _trace=False):
    nc = _get_program()
    in_maps = make_in_maps(x, adj, W, a)
    res = run_bass_kernel_spmd(nc, in_maps, list(range(NCORES)), trace=_trace)
    out = np.concatenate(
        [res.results[c]["out_s"] for c in range(NCORES)], axis=0
    ).astype(np.float32)
    if _trace:
        return out, res
    return out
